# revision 1
# baseline (speedup 1.0000x reference)
"""Trainium2 Bass kernel for the MHA-with-diagonal-softmax module.

Computation (per batch b):
    q = rope(x @ Wq.T), k = rope(x @ Wk.T), v = x @ Wv.T      (per head, DH=128)
    sumexp[s,h] = sum_k exp(q_h[s] . k_h[k] * DH^-0.5)
    diag[s,h]   = q_h[s] . k_h[s] * DH^-0.5
    w = exp(diag) / sumexp
    out = (w * v) @ Wo.T

Sharding: 8 cores = 2 (batch) x 4 (head groups of 4 heads).
Each core computes q/k/v for its 4 heads in transposed [head_dim, seq]
layout, the per-position softmax-diagonal weights, and a partial output
projection (its heads' rows of Wo), written as 2 head-pair partials that
the host sums.

On-chip dtype is fp16 (same PE throughput as bf16, 8x lower rounding
error - matters because exp() amplifies absolute score error), with fp32
PSUM accumulation everywhere.
"""

import numpy as np
from contextlib import ExitStack

# Problem constants (hardcoded per harness contract).
B, S, D, H, DH = 2, 2048, 2048, 16, 128
HPC = 4            # heads per core
NHL = HPC * DH     # 512 local head dims per core
KB = D // 128      # 16 contraction blocks
SB = S // 128      # 16 seq blocks of 128
SC = S // 512      # 4 seq/emb chunks of 512
NCORES = 8

_CACHE = {}


def _build_nc():
    import concourse.bass as bass
    import concourse.tile as tile
    from concourse import bacc, mybir
    from concourse.masks import make_identity

    F16 = mybir.dt.float16
    F32 = mybir.dt.float32
    AF = mybir.ActivationFunctionType
    ALU = mybir.AluOpType
    AX = mybir.AxisListType

    # Bacc (not raw Bass): its compile() splits multi-sem waits into
    # event-semaphore instructions - HW allows at most 1 wait per inst.
    nc = bacc.Bacc("TRN2", target_bir_lowering=False, debug=False)

    xT = nc.dram_tensor("xT", [D, S], F16, kind="ExternalInput").ap()
    wq = nc.dram_tensor("wq", [D, NHL], F16, kind="ExternalInput").ap()
    wk = nc.dram_tensor("wk", [D, NHL], F16, kind="ExternalInput").ap()
    wv = nc.dram_tensor("wv", [D, NHL], F16, kind="ExternalInput").ap()
    wo = nc.dram_tensor("wo", [NHL, D], F16, kind="ExternalInput").ap()
    ropeA = nc.dram_tensor("ropeA", [128, S], F16, kind="ExternalInput").ap()
    ropeB = nc.dram_tensor("ropeB", [128, S], F16, kind="ExternalInput").ap()
    y = nc.dram_tensor("y", [2, S, D], F16, kind="ExternalOutput").ap()

    xT_r = xT.rearrange("(a p) s -> a p s", p=128)
    wq_r = wq.rearrange("(a p) m -> a p m", p=128)
    wk_r = wk.rearrange("(a p) m -> a p m", p=128)
    wv_r = wv.rearrange("(a p) m -> a p m", p=128)
    wo_r = wo.rearrange("(h p) n -> h p n", p=128)

    with tile.TileContext(nc) as tc, ExitStack() as ctx:
        pool = ctx.enter_context(tc.tile_pool(name="sb", bufs=1))
        pp = ctx.enter_context(tc.tile_pool(name="ps", bufs=1, space="PSUM"))

        # ---- constants ----
        ra = pool.tile([128, S], F16, name="ra")
        rb = pool.tile([128, S], F16, name="rb")
        # SWDGE: a wide HWDGE DMA fans out over several HW queues, and a
        # DVE/ACT consumer then needs one sync-wait per queue, exceeding
        # the instruction's wait-slot budget at compile time.
        nc.gpsimd.dma_start(ra[:, :], ropeA[:, :])
        nc.gpsimd.dma_start(rb[:, :], ropeB[:, :])
        ident = pool.tile([128, 128], F32, name="ident")
        make_identity(nc, ident[:, :])
        onesf = pool.tile([128, 128], F32, name="onesf")
        nc.gpsimd.memset(onesf[:, :], 1.0)
        ones1 = pool.tile([128, 128], F16, name="ones1")
        nc.gpsimd.memset(ones1[:, :], 1.0)

        # ---- x resident in SBUF ----
        xsb = pool.tile([128, KB, S], F16, name="xsb")
        for kb in range(KB):
            nc.sync.dma_start(xsb[:, kb, :], xT_r[kb])

        # ---- persistent q/k/v head tiles ([head_dim, seq] layout) ----
        qh = [pool.tile([128, S], F16, name=f"qh{h}") for h in range(HPC)]
        kh = [pool.tile([128, S], F16, name=f"kh{h}") for h in range(HPC)]
        vh = [pool.tile([128, S], F16, name=f"vh{h}") for h in range(HPC)]

        # per-head row vectors live at partition 32*h (engine ops only
        # support start partitions that are multiples of 32)
        ds_diag = pool.tile([128, S], F32, name="ds_diag")
        ds_sum = pool.tile([128, S], F16, name="ds_sum")
        w4 = pool.tile([128, S], F16, name="w4")
        sumf = [pool.tile([128, SB], F32, name=f"sumf{h}") for h in range(HPC)]

        def load_w(src_r, nblk, tag="w"):
            t = pool.tile([128, nblk, 512 * (KB // nblk)], F16, name="wt",
                          tag=tag, bufs=2)
            for i in range(nblk):
                nc.sync.dma_start(t[:, i, :], src_r[i])
            return t

        def proj_chunk(wt, dests, mt, sc):
            # dests[mt][:, sc-chunk] <- (wt[:, :, mt] block).T @ x chunk
            ps = pp.tile([128, 512], F32, name="mmps", tag="mm", bufs=2)
            for kb in range(KB):
                nc.tensor.matmul(
                    ps[:, :],
                    wt[:, kb, mt * 128:(mt + 1) * 128],
                    xsb[:, kb, sc * 512:(sc + 1) * 512],
                    start=(kb == 0), stop=(kb == KB - 1))
            nc.scalar.activation(
                dests[mt][:, sc * 512:(sc + 1) * 512], ps[:, :], AF.Copy)

        def proj(wt, dests):
            for mt in range(HPC):
                for sc in range(SC):
                    proj_chunk(wt, dests, mt, sc)

        def rope(dst):
            # dst (in place): top = te*cos - to*sin ; bottom = te*sin + to*cos
            # ra = [cosT; cosT], rb = [-sinT; sinT]; swap = halves exchanged.
            for c in range(2):
                sl = slice(c * 1024, (c + 1) * 1024)
                # SWDGE (gpsimd) keeps this 1 queue -> 1 sem; a wide HWDGE
                # sbuf->sbuf DMA fans out over many queues and blows the
                # consumer's sync-wait slot budget.
                swp = pool.tile([128, 1024], F16, name="swp", tag="swp", bufs=1)
                nc.gpsimd.dma_start(swp[0:64, :], dst[64:128, sl])
                nc.gpsimd.dma_start(swp[64:128, :], dst[0:64, sl])
                u = pool.tile([128, 1024], F16, name="u", tag="sc", bufs=2)
                nc.vector.tensor_mul(u[:, :], dst[:, sl], ra[:, sl])
                v2 = pool.tile([128, 1024], F16, name="v2", tag="sc", bufs=2)
                nc.vector.tensor_mul(v2[:, :], swp[:, :], rb[:, sl])
                nc.vector.tensor_add(dst[:, sl], u[:, :], v2[:, :])

        def diag(h):
            # ds_diag[32h, s] = sum_m qh[h][m, s] * kh[h][m, s]  (fp32)
            hp = 32 * h
            for c in range(2):
                sl = slice(c * 1024, (c + 1) * 1024)
                pr = pool.tile([128, 1024], F32, name="pr", tag="pr", bufs=1)
                nc.vector.tensor_mul(pr[:, :], qh[h][:, sl], kh[h][:, sl])
                for cc in range(2):
                    dps = pp.tile([128, 512], F32, name="dps", tag="sm", bufs=2)
                    nc.tensor.matmul(dps[:, :], onesf[:, :],
                                     pr[:, cc * 512:(cc + 1) * 512],
                                     start=True, stop=True)
                    o = (2 * c + cc) * 512
                    nc.scalar.activation(ds_diag[hp:hp + 1, o:o + 512],
                                         dps[hp:hp + 1, :], AF.Copy)

        # ====== phase 2 pieces ======
        def scores_sq(h, sq, fillers):
            """One query block: 4 score MMs into a 4-bank psum tile, one wide
            exp with fused row-sum. `fillers` = list of callables emitting
            independent PE work, interleaved so the in-order PE queue always
            has runnable matmuls while ACT drains the exp (keeps HAM warm)."""
            sps = pp.tile([128, S], F32, name="sps", tag="sco", bufs=1)
            for ck in range(SC):
                nc.tensor.matmul(sps[:, ck * 512:(ck + 1) * 512],
                                 qh[h][:, sq * 128:(sq + 1) * 128],
                                 kh[h][:, ck * 512:(ck + 1) * 512],
                                 start=True, stop=True)
            for f in fillers:
                f()
            ex = pool.tile([128, S], F16, name="ex", tag="ex", bufs=1)
            nc.scalar.activation(ex[:, :], sps[:, :], AF.Exp,
                                 accum_out=sumf[h][:, sq:sq + 1])

        def head_sum_tail(h):
            # recip -> transpose -> [1, S] row of ds_sum
            rs = pool.tile([128, SB], F32, name="rs", tag="rs", bufs=2)
            nc.vector.reciprocal(rs[:, :], sumf[h][:, :])
            tps = pp.tile([16, 128], F32, name="tps", tag="sm", bufs=2)
            nc.tensor.transpose(tps[:, :], rs[:, :], ident[:, :])
            st = pool.tile([16, 128], F16, name="st", tag="st", bufs=2)
            nc.vector.tensor_copy(st[:, :], tps[:, :])
            nc.gpsimd.dma_start(ds_sum[32 * h:32 * h + 1, :], st[:, :])

        def pair_head(p):
            # w = exp(diag) * recip(sumexp); attn = w (bcast) * v, into kh
            h0, h1 = 2 * p, 2 * p + 1
            expd = pool.tile([128, S], F16, name="expd", tag="expd", bufs=2)
            for h in (h0, h1):
                hp = 32 * h
                nc.scalar.activation(expd[hp:hp + 1, :], ds_diag[hp:hp + 1, :],
                                     AF.Exp)
                nc.vector.tensor_mul(w4[hp:hp + 1, :], expd[hp:hp + 1, :],
                                     ds_sum[hp:hp + 1, :])
                wb = pool.tile([128, S], F16, name="wb", tag="wb", bufs=1)
                for ck in range(SC):
                    bps = pp.tile([128, 512], F32, name="bps", tag="mm", bufs=2)
                    nc.tensor.matmul(bps[:, :], ones1[hp:hp + 1, :],
                                     w4[hp:hp + 1, ck * 512:(ck + 1) * 512],
                                     start=True, stop=True,
                                     tile_position=(hp, 0))
                    nc.vector.tensor_copy(wb[:, ck * 512:(ck + 1) * 512],
                                          bps[:, :])
                nc.vector.tensor_mul(kh[h][:, :], wb[:, :], vh[h][:, :])

        def oproj_group(p, sb, ncx):
            h0, h1 = 2 * p, 2 * p + 1
            ps = pp.tile([128, 512], F32, name="ops", tag="mm", bufs=2)
            for i, h in enumerate((h0, h1)):
                nc.tensor.matmul(
                    ps[:, :], kh[h][:, sb * 128:(sb + 1) * 128],
                    wot[:, h, ncx * 512:(ncx + 1) * 512],
                    start=(i == 0), stop=(i == 1))
            yt = pool.tile([128, 512], F16, name="yt", tag="yt", bufs=2)
            nc.vector.tensor_copy(yt[:, :], ps[:, :])
            nc.sync.dma_start(
                y[p, sb * 128:(sb + 1) * 128,
                  ncx * 512:(ncx + 1) * 512], yt[:, :])

        # ================= emission =================
        # dense PE phase: K and Q projections + rope + diag
        wkt = load_w(wk_r, KB)
        wqt = load_w(wq_r, KB)
        proj(wkt, kh)
        for h in range(HPC):
            rope(kh[h])
        proj(wqt, qh)
        for h in range(HPC):
            rope(qh[h])
            diag(h)
        # wv reuses wk's slot, wo reuses wq's slot (tag bufs=2)
        wvt = load_w(wv_r, KB)
        wot = load_w(wo_r, HPC)

        # scores streams, with independent matmul work as filler:
        #   head 0/1 slots <- V projection chunks (16 groups of 16 MMs)
        #   head 2 slots   <- pair-0 output projection (64 groups of 2 MMs)
        #   head 3 slots   <- none available (paced by exp)
        vfill = [(mt, sc) for mt in range(HPC) for sc in range(SC)]
        for sq in range(SB):
            f = []
            if sq % 2 == 0 and vfill:
                mt, sc = vfill.pop(0)
                f.append(lambda mt=mt, sc=sc: proj_chunk(wvt, vh, mt, sc))
            scores_sq(0, sq, f)
        head_sum_tail(0)
        for sq in range(SB):
            f = []
            if sq % 2 == 0 and vfill:
                mt, sc = vfill.pop(0)
                f.append(lambda mt=mt, sc=sc: proj_chunk(wvt, vh, mt, sc))
            scores_sq(1, sq, f)
        head_sum_tail(1)
        pair_head(0)
        ofill = [(sb, ncx) for sb in range(SB) for ncx in range(SC)]
        for sq in range(SB):
            f = []
            for _ in range(4):
                if ofill:
                    sb, ncx = ofill.pop(0)
                    f.append(lambda sb=sb, ncx=ncx: oproj_group(0, sb, ncx))
            scores_sq(2, sq, f)
        head_sum_tail(2)
        for sq in range(SB):
            f = []
            if ofill:
                sb, ncx = ofill.pop(0)
                f.append(lambda sb=sb, ncx=ncx: oproj_group(0, sb, ncx))
            scores_sq(3, sq, f)
        head_sum_tail(3)
        for sb, ncx in ofill:
            oproj_group(0, sb, ncx)
        pair_head(1)
        for sb in range(SB):
            for ncx in range(SC):
                oproj_group(1, sb, ncx)

    nc.compile()
    return nc


def _get_nc():
    if "nc" not in _CACHE:
        _CACHE["nc"] = _build_nc()
    return _CACHE["nc"]


_PERM = np.concatenate([np.arange(0, DH, 2), np.arange(1, DH, 2)])


def _host_inputs(x, rope_cos, rope_sin, Wq, Wk, Wv, Wo):
    """Build the 8 per-core input maps."""
    f16 = np.float16
    cosT = np.ascontiguousarray(np.asarray(rope_cos, np.float32)[0, :, 0, :].T)
    sinT = np.ascontiguousarray(np.asarray(rope_sin, np.float32)[0, :, 0, :].T)
    ra = np.concatenate([cosT, cosT], 0).astype(f16)
    rb = np.concatenate([-sinT, sinT], 0).astype(f16)

    Wq = np.asarray(Wq, np.float32)
    Wk = np.asarray(Wk, np.float32)
    Wv = np.asarray(Wv, np.float32)
    Wo = np.asarray(Wo, np.float32)
    x = np.asarray(x, np.float32)

    xTb = [np.ascontiguousarray(x[b].T).astype(f16) for b in range(B)]
    scale = DH ** -0.5

    in_maps = []
    for core in range(NCORES):
        b, g = divmod(core, HPC)
        hs = g * HPC
        rows = np.concatenate(
            [h * DH + _PERM for h in range(hs, hs + HPC)])      # deinterleave
        rows_v = np.arange(hs * DH, (hs + HPC) * DH)
        in_maps.append({
            "xT": xTb[b],
            "wq": np.ascontiguousarray((Wq[rows] * scale).T).astype(f16),
            "wk": np.ascontiguousarray(Wk[rows].T).astype(f16),
            "wv": np.ascontiguousarray(Wv[rows_v].T).astype(f16),
            "wo": np.ascontiguousarray(Wo[:, rows_v].T).astype(f16),
            "ropeA": ra,
            "ropeB": rb,
        })
    return in_maps


def kernel(x, rope_cos, rope_sin, Wq, Wk, Wv, Wo, _trace=False, _trace_cores=None):
    from concourse.bass_utils import run_bass_kernel_spmd

    nc = _get_nc()
    in_maps = _host_inputs(x, rope_cos, rope_sin, Wq, Wk, Wv, Wo)
    res = run_bass_kernel_spmd(nc, in_maps, list(range(NCORES)),
                               trace=_trace, trace_cores=_trace_cores)
    _CACHE["last_result"] = res

    out = np.zeros((B, S, D), np.float32)
    for core in range(NCORES):
        b = core // HPC
        out[b] += res.results[core]["y"].astype(np.float32).sum(axis=0)
    return out



# revision 7
# speedup vs baseline: 1.2527x; 1.2527x over previous
"""Trainium2 Bass kernel for the MHA-with-diagonal-softmax module.

Computation (per batch b):
    q = rope(x @ Wq.T), k = rope(x @ Wk.T), v = x @ Wv.T      (per head, DH=128)
    sumexp[s,h] = sum_k exp(q_h[s] . k_h[k] * DH^-0.5)
    diag[s,h]   = q_h[s] . k_h[s] * DH^-0.5
    w = exp(diag) / sumexp
    out = (w * v) @ Wo.T

Sharding: 8 cores = 2 (batch) x 4 (head groups of 4 heads).
Each core computes q/k/v for its 4 heads in transposed [head_dim, seq]
layout, the per-position softmax-diagonal weights, and a partial output
projection (its heads' rows of Wo), written as 2 head-pair partials that
the host sums.

Performance structure: the kernel is one near-continuous PE matmul
stream.  K proj and Q proj (head 0) run first (input DMAs are chunked
seq-major so compute starts after ~2.5 MB instead of 10 MB).  All
remaining matmul work (Q proj heads 1-3, diag, V proj, output proj of
pair 0) lives in a filler queue that is drained between score blocks,
so the per-block exp() on the scalar engine never stalls the PE.  Score
PSUM is 2x[128,1024] double-buffered.  exp(diag) rows are precomputed
the moment each head's diag exists, shortening the pair transform
chains; a reserve of output-proj fillers covers the pair-1 transform
window before the tail.

On-chip dtype is fp16 (same PE throughput as bf16, 8x lower rounding
error - matters because exp() amplifies absolute score error), with fp32
PSUM accumulation everywhere.
"""

import numpy as np
from contextlib import ExitStack
from collections import deque

# Problem constants (hardcoded per harness contract).
B, S, D, H, DH = 2, 2048, 2048, 16, 128
HPC = 4            # heads per core
NHL = HPC * DH     # 512 local head dims per core
KB = D // 128      # 16 contraction blocks
SB = S // 128      # 16 seq blocks of 128
SC = S // 512      # 4 seq/emb chunks of 512
NCORES = 8

_CACHE = {}


def _build_nc():
    import concourse.bass as bass
    import concourse.tile as tile
    from concourse import bacc, mybir
    from concourse.masks import make_identity

    F16 = mybir.dt.float16
    F32 = mybir.dt.float32
    AF = mybir.ActivationFunctionType

    # Bacc (not raw Bass): its compile() splits multi-sem waits into
    # event-semaphore instructions - HW allows at most 1 wait per inst.
    nc = bacc.Bacc("TRN2", target_bir_lowering=False, debug=False)

    xT = nc.dram_tensor("xT", [D, S], F16, kind="ExternalInput").ap()
    wq = nc.dram_tensor("wq", [D, NHL], F16, kind="ExternalInput").ap()
    wk = nc.dram_tensor("wk", [D, NHL], F16, kind="ExternalInput").ap()
    wv = nc.dram_tensor("wv", [D, NHL], F16, kind="ExternalInput").ap()
    wo = nc.dram_tensor("wo", [NHL, D], F16, kind="ExternalInput").ap()
    ropeA = nc.dram_tensor("ropeA", [128, S], F16, kind="ExternalInput").ap()
    ropeB = nc.dram_tensor("ropeB", [128, S], F16, kind="ExternalInput").ap()
    y = nc.dram_tensor("y", [2, S, D], F16, kind="ExternalOutput").ap()

    # [kb, sc, 128, 512] view of x for chunked loads
    xT_c = xT.rearrange("(a p) (c w) -> a c p w", p=128, w=512)
    wq_r = wq.rearrange("(a p) m -> a p m", p=128)
    wk_r = wk.rearrange("(a p) m -> a p m", p=128)
    wv_r = wv.rearrange("(a p) m -> a p m", p=128)
    wo_r = wo.rearrange("(h p) n -> h p n", p=128)

    with tile.TileContext(nc) as tc, ExitStack() as ctx:
        pool = ctx.enter_context(tc.tile_pool(name="sb", bufs=1))
        pp = ctx.enter_context(tc.tile_pool(name="ps", bufs=1, space="PSUM"))

        # ---- constants ----
        ra = pool.tile([128, S], F16, name="ra")
        rb = pool.tile([128, S], F16, name="rb")
        # SWDGE: a wide HWDGE DMA fans out over several HW queues, and a
        # DVE/ACT consumer then needs one sync-wait per queue, exceeding
        # the instruction's wait-slot budget at compile time.
        nc.gpsimd.dma_start(ra[:, :], ropeA[:, :])
        nc.gpsimd.dma_start(rb[:, :], ropeB[:, :])
        ident = pool.tile([128, 128], F32, name="ident")
        make_identity(nc, ident[:, :])
        ones1 = pool.tile([128, 128], F16, name="ones1")
        nc.gpsimd.memset(ones1[:, :], 1.0)

        # ---- weight + x loads, ordered by first use ----
        def load_w(src_r, nblk, tag="w"):
            t = pool.tile([128, nblk, 512 * (KB // nblk)], F16, name="wt",
                          tag=tag, bufs=2)
            for i in range(nblk):
                nc.sync.dma_start(t[:, i, :], src_r[i])
            return t

        wkt = load_w(wk_r, KB)
        xsb = pool.tile([128, KB, S], F16, name="xsb")

        # sc-major so K proj (which consumes one sc-chunk across all kb)
        # can start after the first chunk instead of after all of x.
        def load_x_sc(sc):
            for kb in range(KB):
                nc.sync.dma_start(xsb[:, kb, sc * 512:(sc + 1) * 512],
                                  xT_c[kb, sc])

        load_x_sc(0)
        wqt = load_w(wq_r, KB)
        for sc in range(1, SC):
            load_x_sc(sc)

        # ---- persistent q/k/v head tiles ([head_dim, seq] layout) ----
        qh = [pool.tile([128, S], F16, name=f"qh{h}") for h in range(HPC)]
        kh = [pool.tile([128, S], F16, name=f"kh{h}") for h in range(HPC)]
        vh = [pool.tile([128, S], F16, name=f"vh{h}") for h in range(HPC)]

        # per-head row vectors live at partition 32*h (engine ops only
        # support start partitions that are multiples of 32)
        ds_diag = pool.tile([128, S], F32, name="ds_diag")
        expd = pool.tile([128, S], F16, name="expd")
        ds_sum = pool.tile([128, S], F16, name="ds_sum")
        w4 = pool.tile([128, S], F16, name="w4")
        # col half*SB+sq = sumexp over the keys in that 1024-half
        sumf = [pool.tile([128, 2 * SB], F32, name=f"sumf{h}")
                for h in range(HPC)]

        # ================= building blocks =================
        def proj_mms(wt, mt, sc, ps, kbs):
            for kb in kbs:
                nc.tensor.matmul(
                    ps[:, :],
                    wt[:, kb, mt * 128:(mt + 1) * 128],
                    xsb[:, kb, sc * 512:(sc + 1) * 512],
                    start=(kb == 0), stop=(kb == KB - 1))

        def proj_chunk(wt, dests, mt, sc, evac="act"):
            # dests[mt][:, sc-chunk] <- (wt[:, :, mt] block).T @ x chunk
            ps = pp.tile([128, 512], F32, name="mmps", tag="mm", bufs=2)
            proj_mms(wt, mt, sc, ps, range(KB))
            dst = dests[mt][:, sc * 512:(sc + 1) * 512]
            if evac == "act":
                nc.scalar.activation(dst, ps[:, :], AF.Copy)
            else:
                nc.vector.tensor_copy(dst, ps[:, :])

        def proj_unit(wt, dests, mt, sc, evac="act"):
            # one chunk as an atomic filler unit.  Atomicity matters: the
            # 16-matmul PSUM accumulation group must not interleave with
            # another 'mm'-tag allocation (same-bank reuse would clear
            # has_written mid-group).
            return (3460, lambda: proj_chunk(wt, dests, mt, sc, evac))

        def rope(dst):
            # dst (in place): top = te*cos - to*sin ; bottom = te*sin + to*cos
            # ra = [cosT; cosT], rb = [-sinT; sinT]; swap = halves exchanged.
            for c in range(2):
                sl = slice(c * 1024, (c + 1) * 1024)
                # SWDGE (gpsimd) keeps this 1 queue -> 1 sem; a wide HWDGE
                # sbuf->sbuf DMA fans out over many queues and blows the
                # consumer's sync-wait slot budget.
                swp = pool.tile([128, 1024], F16, name="swp", tag="swp", bufs=2)
                nc.gpsimd.dma_start(swp[0:64, :], dst[64:128, sl])
                nc.gpsimd.dma_start(swp[64:128, :], dst[0:64, sl])
                u = pool.tile([128, 1024], F16, name="u", tag="sc", bufs=2)
                nc.vector.tensor_mul(u[:, :], dst[:, sl], ra[:, sl])
                v2 = pool.tile([128, 1024], F16, name="v2", tag="sc", bufs=2)
                nc.vector.tensor_mul(v2[:, :], swp[:, :], rb[:, sl])
                nc.vector.tensor_add(dst[:, sl], u[:, :], v2[:, :])

        def diag_unit(h, c):
            # ds_diag[32h, c-half] = per-position q.k (fp16 products,
            # fp32 psum accumulation via ones-matmul column sum)
            hp = 32 * h
            sl = slice(c * 1024, (c + 1) * 1024)
            pr = pool.tile([128, 1024], F16, name="pr", tag="pr", bufs=2)
            nc.vector.tensor_mul(pr[:, :], qh[h][:, sl], kh[h][:, sl])
            for cc in range(2):
                dps = pp.tile([128, 512], F32, name="dps", tag="mm", bufs=2)
                nc.tensor.matmul(dps[:, :], ones1[:, :],
                                 pr[:, cc * 512:(cc + 1) * 512],
                                 start=True, stop=True)
                o = (2 * c + cc) * 512
                nc.scalar.activation(ds_diag[hp:hp + 1, o:o + 512],
                                     dps[hp:hp + 1, :], AF.Copy)

        def expd_row(h):
            hp = 32 * h
            nc.scalar.activation(expd[hp:hp + 1, :], ds_diag[hp:hp + 1, :],
                                 AF.Exp)

        def sco_block(h, sq, half):
            # one 128x1024 score tile -> exp with fused key-axis row sum
            sps = pp.tile([128, 1024], F32, name="sps", tag="sco", bufs=2)
            for cc in range(2):
                o = half * 1024 + cc * 512
                nc.tensor.matmul(sps[:, cc * 512:(cc + 1) * 512],
                                 qh[h][:, sq * 128:(sq + 1) * 128],
                                 kh[h][:, o:o + 512],
                                 start=True, stop=True)
            ex = pool.tile([128, 1024], F16, name="ex", tag="ex", bufs=2)
            col = half * SB + sq
            nc.scalar.activation(ex[:, :], sps[:, :], AF.Exp,
                                 accum_out=sumf[h][:, col:col + 1])

        def head_sum_tail(h):
            # halves-add -> recip -> transpose -> [1, S] row of ds_sum
            sums = pool.tile([128, SB], F32, name="sums", tag="sums", bufs=2)
            nc.vector.tensor_add(sums[:, :], sumf[h][:, 0:SB],
                                 sumf[h][:, SB:2 * SB])
            rs = pool.tile([128, SB], F32, name="rs", tag="rs", bufs=2)
            nc.vector.reciprocal(rs[:, :], sums[:, :])
            tps = pp.tile([128, 512], F32, name="tps", tag="mm", bufs=2)
            nc.tensor.transpose(tps[0:SB, 0:128], rs[:, :], ident[:, :])
            st = pool.tile([16, 128], F16, name="st", tag="st", bufs=2)
            nc.vector.tensor_copy(st[:, :], tps[0:SB, 0:128])
            nc.gpsimd.dma_start(ds_sum[32 * h:32 * h + 1, :], st[:, :])

        def head_weights(h, fill):
            # w = exp(diag) * recip(sumexp); attn = w (bcast) * v, into kh.
            # expd row precomputed; broadcast matmul output is consumed
            # directly from PSUM by the DVE multiply (no staging copy).
            hp = 32 * h
            nc.vector.tensor_mul(w4[hp:hp + 1, :], expd[hp:hp + 1, :],
                                 ds_sum[hp:hp + 1, :])
            for ck in range(SC):
                bps = pp.tile([128, 512], F32, name="bps", tag="op", bufs=2)
                nc.tensor.matmul(bps[:, :], ones1[hp:hp + 1, :],
                                 w4[hp:hp + 1, ck * 512:(ck + 1) * 512],
                                 start=True, stop=True,
                                 tile_position=(hp, 0))
                sl = slice(ck * 512, (ck + 1) * 512)
                nc.vector.tensor_mul(kh[h][:, sl], bps[:, :], vh[h][:, sl])
                fill(220)

        def oproj_group(p, sb, ncx):
            h0, h1 = 2 * p, 2 * p + 1
            wot = wot_box[0]
            ps = pp.tile([128, 512], F32, name="ops", tag="op", bufs=2)
            for i, h in enumerate((h0, h1)):
                nc.tensor.matmul(
                    ps[:, :], kh[h][:, sb * 128:(sb + 1) * 128],
                    wot[:, h, ncx * 512:(ncx + 1) * 512],
                    start=(i == 0), stop=(i == 1))
            yt = pool.tile([128, 512], F16, name="yt", tag="yt", bufs=3)
            nc.vector.tensor_copy(yt[:, :], ps[:, :])
            nc.sync.dma_start(
                y[p, sb * 128:(sb + 1) * 128,
                  ncx * 512:(ncx + 1) * 512], yt[:, :])

        # ================= filler queue =================
        # Units of (pe_cost_ns, emit_fn), drained between score blocks to
        # keep the PE streaming while ACT runs the exps.
        fillers = deque()
        fill_debt = [0.0]

        def fill(budget):
            fill_debt[0] += budget
            while fillers and fillers[0][0] <= fill_debt[0]:
                cost, fn = fillers.popleft()
                fn()
                fill_debt[0] -= cost

        def drain(dq):
            while dq:
                _, fn = dq.popleft()
                fn()

        # ================= emission =================
        # dense PE lead-in: K projection + Q projection head 0, sc-outer
        # so the chunked x DMAs are consumed in arrival order.  In the
        # last sc-iteration the q-head-0 chunk goes first and each rope
        # is emitted the moment its tensor completes, so the DVE ropes
        # overlap the remaining PE chunks and score block 0 starts with
        # at most ~1us of rope wait.
        for sc in range(SC - 1):
            for mt in range(HPC):
                proj_chunk(wkt, kh, mt, sc)
            proj_chunk(wqt, qh, 0, sc)
        proj_chunk(wqt, qh, 0, SC - 1)
        rope(qh[0])
        for mt in range(HPC):
            proj_chunk(wkt, kh, mt, SC - 1)
            rope(kh[mt])
        # wv reuses wk's slot, wo reuses wq's slot (tag bufs=2); the loads
        # self-delay on the WAR semaphore of the previous consumer.
        wvt = load_w(wv_r, KB)

        # Emission-time progress flags for the force-drain guards below
        # (all bookkeeping is emission-time python, fully deterministic).
        q_ready = [True] + [False] * (HPC - 1)
        v_done = [0] * HPC
        e_done = [False] * HPC

        def mark(fn, after):
            def wrapped():
                fn()
                after()
            return wrapped

        # queue: diag/expd h0, Q proj h1 (+rope/diag/expd), V proj h0/h1,
        # Q proj h2/h3, V proj h2/h3; pair-0 output proj appended later.
        for c in range(2):
            fillers.append((440, lambda c=c: diag_unit(0, c)))
        fillers.append(
            (0, mark(lambda: expd_row(0),
                     lambda: e_done.__setitem__(0, True))))
        wot_box = []

        def queue_qhead(hq):
            for sc in range(SC):
                fillers.append(proj_unit(wqt, qh, hq, sc))
            fillers.append(
                (0, mark(lambda hq=hq: rope(qh[hq]),
                         lambda hq=hq: q_ready.__setitem__(hq, True))))
            for c in range(2):
                fillers.append((440, lambda hq=hq, c=c: diag_unit(hq, c)))
            fillers.append(
                (0, mark(lambda hq=hq: expd_row(hq),
                         lambda hq=hq: e_done.__setitem__(hq, True))))

        def queue_vhead(mt):
            for sc in range(SC):
                fillers.append(
                    (3460, mark(
                        lambda mt=mt, sc=sc:
                        proj_chunk(wvt, vh, mt, sc, evac="dve"),
                        lambda mt=mt:
                        v_done.__setitem__(mt, v_done[mt] + 1))))

        queue_qhead(1)
        queue_vhead(0)
        queue_vhead(1)
        queue_qhead(2)
        queue_qhead(3)
        # wo load directly after the last wqt-consuming unit
        fillers.append((0, lambda: wot_box.append(load_w(wo_r, HPC))))
        queue_vhead(2)
        queue_vhead(3)

        FILL_A = 1150   # ns of filler per score block, first half
        FILL_B = 760    # second half (oproj-0 units are cheaper)

        reserve = deque()

        def rfill(budget):
            fill_debt[0] += budget
            while reserve and reserve[0][0] <= fill_debt[0]:
                cost, fn = reserve.popleft()
                fn()
                fill_debt[0] -= cost

        def force(cond):
            # pop fillers (in order) until an emission-order precondition
            # holds; keeps DVE/PE FIFO deps acyclic regardless of budgets
            while fillers and not cond():
                _, fn = fillers.popleft()
                fn()
            assert cond()

        def emit_head_blocks(h, per_block):
            force(lambda: q_ready[h])
            for sq in range(SB):
                for half in range(2):
                    sco_block(h, sq, half)
                    fill(per_block)

        def pair_tail(h, f):
            # per-head: sumexp row, then w + attn, right after the head's
            # last score block; fillers cover the serial chain latency
            f(1500)
            head_sum_tail(h)
            force(lambda: v_done[h] >= SC and e_done[h])
            f(1200)
            head_weights(h, f)

        emit_head_blocks(0, FILL_A)
        pair_tail(0, fill)
        emit_head_blocks(1, FILL_A)
        pair_tail(1, fill)

        # pair-0 output projection becomes available (16 units reserved
        # as PE cover for the head-3 transform window)
        oq = [(s_, n_) for s_ in range(SB) for n_ in range(SC)]
        for i, (s_, n_) in enumerate(oq):
            unit = (432, lambda s_=s_, n_=n_: oproj_group(0, s_, n_))
            (reserve if i >= len(oq) - 16 else fillers).append(unit)

        emit_head_blocks(2, FILL_B)
        pair_tail(2, fill)
        emit_head_blocks(3, FILL_B)
        drain(fillers)
        pair_tail(3, rfill)
        drain(reserve)

        # ---- tail: pair 1 output projection ----
        for sb in range(SB):
            for ncx in range(SC):
                oproj_group(1, sb, ncx)

    nc.compile()
    return nc


def _get_nc():
    if "nc" not in _CACHE:
        _CACHE["nc"] = _build_nc()
    return _CACHE["nc"]


_PERM = np.concatenate([np.arange(0, DH, 2), np.arange(1, DH, 2)])


def _host_inputs(x, rope_cos, rope_sin, Wq, Wk, Wv, Wo):
    """Build the 8 per-core input maps."""
    f16 = np.float16
    cosT = np.ascontiguousarray(np.asarray(rope_cos, np.float32)[0, :, 0, :].T)
    sinT = np.ascontiguousarray(np.asarray(rope_sin, np.float32)[0, :, 0, :].T)
    ra = np.concatenate([cosT, cosT], 0).astype(f16)
    rb = np.concatenate([-sinT, sinT], 0).astype(f16)

    Wq = np.asarray(Wq, np.float32)
    Wk = np.asarray(Wk, np.float32)
    Wv = np.asarray(Wv, np.float32)
    Wo = np.asarray(Wo, np.float32)
    x = np.asarray(x, np.float32)

    xTb = [np.ascontiguousarray(x[b].T).astype(f16) for b in range(B)]
    scale = DH ** -0.5

    in_maps = []
    for core in range(NCORES):
        b, g = divmod(core, HPC)
        hs = g * HPC
        rows = np.concatenate(
            [h * DH + _PERM for h in range(hs, hs + HPC)])      # deinterleave
        rows_v = np.arange(hs * DH, (hs + HPC) * DH)
        in_maps.append({
            "xT": xTb[b],
            "wq": np.ascontiguousarray((Wq[rows] * scale).T).astype(f16),
            "wk": np.ascontiguousarray(Wk[rows].T).astype(f16),
            "wv": np.ascontiguousarray(Wv[rows_v].T).astype(f16),
            "wo": np.ascontiguousarray(Wo[:, rows_v].T).astype(f16),
            "ropeA": ra,
            "ropeB": rb,
        })
    return in_maps


def kernel(x, rope_cos, rope_sin, Wq, Wk, Wv, Wo, _trace=False, _trace_cores=None):
    from concourse.bass_utils import run_bass_kernel_spmd

    nc = _get_nc()
    in_maps = _host_inputs(x, rope_cos, rope_sin, Wq, Wk, Wv, Wo)
    res = run_bass_kernel_spmd(nc, in_maps, list(range(NCORES)),
                               trace=_trace, trace_cores=_trace_cores)
    _CACHE["last_result"] = res

    out = np.zeros((B, S, D), np.float32)
    for core in range(NCORES):
        b = core // HPC
        out[b] += res.results[core]["y"].astype(np.float32).sum(axis=0)
    return out
